# revision 30
# baseline (speedup 1.0000x reference)
"""BERT attention layer (nn_BertAttention) as a Bass/Tile kernel on 8 trn2 cores.

Sharding: data-parallel over batch (B=8 -> 1 batch element per core, no
collectives). Each core computes QKV projections, per-head attention,
masked output projection, residual + LayerNorm for its batch element.

Layout strategy (per core, S=1024, D=768, H=12, HD=64):
  - hidden^T (x_t [D, S]) feeds projections; Q,K produced transposed
    (q_t/k_t [D, S], head pair per 128-partition chunk), V natural [S, D].
  - scores computed transposed: s_t[k, q] = K @ Q^T per head. ACT exp
    folds the 1/sqrt(HD) scale and the per-k attention-mask bias.
  - PV: ctx^T[d, q] with e_t as moving operand; a concurrent all-ones
    [128,32] matmul in a disjoint column group produces the softmax
    denominator rows for free.
  - denominator reciprocal: one row is bounced through DRAM into a
    [128, 8] partition spread (so the iterative-divide DVE reciprocal
    touches 8 elems/lane instead of 1024), then broadcast back to
    [64, S] via a zero-stride DRAM read for the normalization multiply.
  - out-projection accumulates all 12 heads (+bias row) into psum per
    q-chunk; residual + LayerNorm fused on DVE with ACT ln/exp for
    rsqrt (keeps a single activation table set for the whole kernel).
  - head_mask is folded into Wv/bv host-side; biases enter as augmented
    rank-1 matmuls (ones x bias-row).
"""

import numpy as np
import ml_dtypes

B, S, D = 8, 1024, 768
H, HD = 12, 64
NCORES = 8
EPS = 1e-12
DI_CH = D // 128   # 6 contraction chunks
S_CH = S // 128    # 8 sequence chunks
PAIRS = H // 2     # 6 head pairs

_CACHE: dict = {}


def _ensure_path():
    import sys
    if "/opt/trn_rl_repo" not in sys.path:
        sys.path.insert(0, "/opt/trn_rl_repo")


def _finalize(ctx, nc, key):
    """Close the TileContext (schedules), compile, cache."""
    ctx.close()
    nc.compile()
    _CACHE[key] = nc
    return nc


def build_program(phases=4):
    """Build (once) the Bass program shared by all cores."""
    key = ("nc", phases)
    if key in _CACHE:
        return _CACHE[key]
    _ensure_path()
    from contextlib import ExitStack
    import concourse.bass as bass
    import concourse.bacc as bacc
    import concourse.mybir as mybir
    import concourse.tile as tile

    F32 = mybir.dt.float32
    BF16 = mybir.dt.bfloat16
    AF = mybir.ActivationFunctionType
    ALU = mybir.AluOpType

    nc = bacc.Bacc("TRN2", target_bir_lowering=False, debug=False)

    xt_d = nc.declare_dram_parameter("xt", [D, S], BF16, isOutput=False)
    xn_d = nc.declare_dram_parameter("xn", [S, D], F32, isOutput=False)
    wqt_d = nc.declare_dram_parameter("wqt", [D, D], BF16, isOutput=False)
    wkt_d = nc.declare_dram_parameter("wkt", [D, D], BF16, isOutput=False)
    wvt_d = nc.declare_dram_parameter("wvt", [D, D], BF16, isOutput=False)
    wot_d = nc.declare_dram_parameter("wot", [D, D], BF16, isOutput=False)
    bq_d = nc.declare_dram_parameter("bqr", [1, D], BF16, isOutput=False)
    bk_d = nc.declare_dram_parameter("bkr", [1, D], BF16, isOutput=False)
    bv_d = nc.declare_dram_parameter("bvr", [1, D], BF16, isOutput=False)
    bo_d = nc.declare_dram_parameter("bor", [1, D], BF16, isOutput=False)
    mask_d = nc.declare_dram_parameter("maskc", [128, S_CH], F32, isOutput=False)
    gb_d = nc.declare_dram_parameter("gammab", [128, D], F32, isOutput=False)
    bb_d = nc.declare_dram_parameter("betab", [128, D], F32, isOutput=False)
    out_d = nc.declare_dram_parameter("out", [S, D], F32, isOutput=True)

    dn_b1 = nc.dram_tensor("dn_b1", [H, S], F32)   # denominator bounce
    dn_b2 = nc.dram_tensor("dn_b2", [H, S], F32)   # reciprocal bounce

    with ExitStack() as ctx:
        tc = ctx.enter_context(tile.TileContext(nc))
        const = ctx.enter_context(tc.tile_pool(name="const", bufs=1))
        wpool = ctx.enter_context(tc.tile_pool(name="w", bufs=1))
        apool = ctx.enter_context(tc.tile_pool(name="act", bufs=1))
        epool = ctx.enter_context(tc.tile_pool(name="e", bufs=10))
        xnpool = ctx.enter_context(tc.tile_pool(name="xn", bufs=2))
        lnpool = ctx.enter_context(tc.tile_pool(name="ln", bufs=2))
        mpool = ctx.enter_context(tc.tile_pool(name="mini", bufs=4))
        rpool = ctx.enter_context(tc.tile_pool(name="rp", bufs=2))
        psum = ctx.enter_context(tc.tile_pool(name="ps", bufs=2, space="PSUM"))

        # ---- constants / small tensors ----
        ones_row = const.tile([1, S], BF16, tag="ones_row")
        nc.vector.memset(ones_row[:], 1.0)
        ones_dup = const.tile([128, 32], BF16, tag="ones_dup")
        nc.vector.memset(ones_dup[:], 1.0)
        zeros_dup = const.tile([128, 32], BF16, tag="zeros_dup")
        nc.vector.memset(zeros_dup[:], 0.0)
        mask_sb = const.tile([128, S_CH], F32, tag="mask")
        nc.sync.dma_start(mask_sb[:], mask_d[:])
        gb_sb = const.tile([128, D], F32, tag="gb")
        nc.sync.dma_start(gb_sb[:], gb_d[:])
        bb_sb = const.tile([128, D], F32, tag="bb")
        nc.sync.dma_start(bb_sb[:], bb_d[:])
        bq_sb = const.tile([1, D], BF16, tag="bq")
        nc.sync.dma_start(bq_sb[:], bq_d[:])
        bk_sb = const.tile([1, D], BF16, tag="bk")
        nc.sync.dma_start(bk_sb[:], bk_d[:])
        bv_sb = const.tile([1, D], BF16, tag="bv")
        nc.sync.dma_start(bv_sb[:], bv_d[:])
        bo_sb = const.tile([1, D], BF16, tag="bo")
        nc.sync.dma_start(bo_sb[:], bo_d[:])

        # ---- bulk input loads ----
        xt_sb = []
        for c in range(DI_CH):
            t = wpool.tile([128, S], BF16, tag=f"xt{c}")
            nc.sync.dma_start(t[:], xt_d[c * 128:(c + 1) * 128, :])
            xt_sb.append(t)
        wq_sb, wk_sb, wv_sb = [], [], []
        for name, dram, lst in (("wq", wqt_d, wq_sb), ("wk", wkt_d, wk_sb),
                                ("wv", wvt_d, wv_sb)):
            for c in range(DI_CH):
                t = wpool.tile([128, D], BF16, tag=f"{name}{c}")
                nc.sync.dma_start(t[:], dram[c * 128:(c + 1) * 128, :])
                lst.append(t)
        # Wo stored per head at partition base 0: all out-projection
        # matmuls then accumulate with a uniform contract base (mixing
        # contract bases 0/64 inside one psum accumulation group faults
        # the device).
        wo_sb = []
        for h in range(H):
            t = wpool.tile([64, D], BF16, tag=f"wo{h}")
            nc.sync.dma_start(t[:], wot_d[h * HD:(h + 1) * HD, :])
            wo_sb.append(t)

        VSPL = [(0, 512), (512, 256)]  # free-dim splits for D=768 outputs

        # ---- phase 1: V projection -> v_sb[s] [128 tokens, 768 dv] ----
        v_sb = []
        for s in range(S_CH):
            vt = apool.tile([128, D], BF16, tag=f"v{s}")
            ps = psum.tile([128, S], F32, tag="big")
            for n0, nsz in VSPL:
                for di in range(DI_CH):
                    nc.tensor.matmul(ps[:, n0:n0 + nsz],
                                     xt_sb[di][:, s * 128:(s + 1) * 128],
                                     wv_sb[di][:, n0:n0 + nsz],
                                     start=(di == 0), stop=False)
                nc.tensor.matmul(ps[:, n0:n0 + nsz],
                                 ones_row[0:1, 0:128],
                                 bv_sb[0:1, n0:n0 + nsz],
                                 start=False, stop=True)
            nc.vector.tensor_copy(vt[:], ps[:, 0:D])
            v_sb.append(vt)

        if phases < 2:
            for s in range(S_CH):
                o = lnpool.tile([128, D], F32, tag="o")
                nc.vector.tensor_copy(o[:], v_sb[s][:])
                nc.sync.dma_start(out_d[s * 128:(s + 1) * 128, :], o[:])
            return _finalize(ctx, nc, key)

        # ---- phase 2: Q/K projections (transposed layout) ----
        q_sb, k_sb = [], []
        for which, w_sb, b_sb, lst in (("q", wq_sb, bq_sb, q_sb),
                                       ("k", wk_sb, bk_sb, k_sb)):
            for c in range(DI_CH):
                t = apool.tile([128, S], BF16, tag=f"{which}t{c}")
                ps = psum.tile([128, S], F32, tag="big")
                for n in range(2):
                    nsl = slice(n * 512, (n + 1) * 512)
                    for di in range(DI_CH):
                        nc.tensor.matmul(ps[:, nsl],
                                         w_sb[di][:, c * 128:(c + 1) * 128],
                                         xt_sb[di][:, nsl],
                                         start=(di == 0), stop=False)
                    nc.tensor.matmul(ps[:, nsl],
                                     b_sb[0:1, c * 128:(c + 1) * 128],
                                     ones_row[0:1, nsl],
                                     start=False, stop=True)
                nc.vector.tensor_copy(t[:], ps[:])
                lst.append(t)

        if phases < 3:
            for c in range(DI_CH):
                o = lnpool.tile([128, D], F32, tag="o2")
                nc.vector.tensor_copy(o[:], q_sb[c][:, 0:D])
                nc.sync.dma_start(out_d[c * 128:(c + 1) * 128, :], o[:])
            return _finalize(ctx, nc, key)

        # ---- phase 3: attention, head pair per chunk ----
        ctx_sb = []
        for h in range(H):
            ctx_t = apool.tile([64, S], BF16, tag=f"ctx{h}", name=f"ctx{h}")
            ctx_sb.append(ctx_t)
        for p in range(PAIRS):
            for half in range(2):  # q/k rows of the pair chunk per head
                h = 2 * p + half
                rlo, rhi = (0, 64) if half == 0 else (64, 128)
                clo, chi = 0, 64   # ctx rows (col groups 0,1)
                dlo = 96           # denominator rows (col group 3)
                e_tiles = []
                for c in range(S_CH):
                    ss = psum.tile([128, S], F32, tag="ss")
                    for n in range(2):
                        nsl = slice(n * 512, (n + 1) * 512)
                        nc.tensor.matmul(ss[:, nsl],
                                         k_sb[p][rlo:rhi, c * 128:(c + 1) * 128],
                                         q_sb[p][rlo:rhi, nsl],
                                         start=True, stop=True)
                    e = epool.tile([128, S], BF16, tag="e")
                    nc.scalar.activation(e[:], ss[:], AF.Exp,
                                         bias=mask_sb[:, c:c + 1], scale=0.125)
                    e_tiles.append(e)
                # ctx rows and denominator rows share psum banks. They must
                # behave as one has_written accumulation group per bank:
                # a zeroing matmul primes the denominator rows, the first
                # ctx matmul carries start (whole-bank bit clear), the last
                # denominator matmul carries stop. PE executes in order;
                # sync=False edges pin the schedule order.
                from concourse.tile_rust import add_dep_helper
                ct = psum.tile([128, S], F32, tag="big")
                zmm = []
                for n in range(2):
                    nsl = slice(n * 512, (n + 1) * 512)
                    z = nc.tensor.matmul(ct[dlo:dlo + 32, nsl], zeros_dup[:],
                                         e_tiles[0][:, nsl],
                                         start=True, stop=False,
                                         tile_position=(0, dlo),
                                         skip_group_check=True)
                    zmm.append(z)
                def mk_ctx(c, n, nsl, st, sp):
                    mmv = nc.tensor.matmul(ct[clo:chi, nsl],
                                           v_sb[c][:, h * HD:(h + 1) * HD],
                                           e_tiles[c][:, nsl],
                                           start=st, stop=sp,
                                           skip_group_check=True)
                    if st:
                        add_dep_helper(mmv.ins, zmm[n].ins, sync=False,
                                       reason="ctx start after denom zero-prime")
                    return mmv

                def mk_dn(c, n, nsl):
                    return nc.tensor.matmul(ct[dlo:dlo + 32, nsl],
                                            ones_dup[:],
                                            e_tiles[c][:, nsl],
                                            start=False, stop=False,
                                            tile_position=(0, dlo),
                                            skip_group_check=True)

                for c in range(S_CH):
                    st, sp = (c == 0), (c == S_CH - 1)
                    for n in range(2):
                        nsl = slice(n * 512, (n + 1) * 512)
                        # creation order = PE issue order for same-bank
                        # accumulation: ctx start first; last denom before
                        # the ctx stop that clears the group
                        if sp:
                            mmd = mk_dn(c, n, nsl)
                            mmv = mk_ctx(c, n, nsl, st, sp)
                            add_dep_helper(mmv.ins, mmd.ins, sync=False,
                                           reason="ctx stop after last denom")
                        else:
                            mmv = mk_ctx(c, n, nsl, st, sp)
                            mmd = mk_dn(c, n, nsl)
                            add_dep_helper(mmd.ins, mmv.ins, sync=False,
                                           reason="denom shares psum group with ctx")
                # denominator -> DRAM -> [128, 8] spread -> reciprocal ->
                # DRAM -> zero-stride broadcast [64, S]
                dcp = rpool.tile([1, S], F32, tag="dcp", bufs=1)
                nc.vector.tensor_copy(dcp[:], ct[dlo:dlo + 1, :])
                nc.sync.dma_start(dn_b1.ap()[h:h + 1, :], dcp[:])
                rs = rpool.tile([128, S // 128], F32, tag="rs")
                nc.sync.dma_start(
                    rs[:], dn_b1.ap()[h:h + 1, :].rearrange(
                        "one (p j) -> (one p) j", p=128))
                rc = rpool.tile([128, S // 128], F32, tag="rc")
                nc.vector.reciprocal(rc[:], rs[:])
                nc.sync.dma_start(
                    dn_b2.ap()[h:h + 1, :].rearrange(
                        "one (p j) -> (one p) j", p=128), rc[:])
                bc = rpool.tile([64, S], F32, tag="bc")
                nc.sync.dma_start(bc[:], bass.AP(dn_b2, h * S, [[0, 64], [1, S]]))
                nc.vector.tensor_tensor(ctx_sb[h][:], ct[clo:chi, :],
                                        bc[:], ALU.mult)

        if phases < 4:
            for h in range(H):
                o = lnpool.tile([64, S], F32, tag="octx")
                nc.vector.tensor_copy(o[:], ctx_sb[h][:])
                nc.sync.dma_start(out_d[h * 64:(h + 1) * 64, 0:D], o[:, 0:D])
            return _finalize(ctx, nc, key)

        # ---- phase 4: output projection + residual + LayerNorm ----
        for s in range(S_CH):
            ps = psum.tile([128, S], F32, tag="big")
            for n0, nsz in VSPL:
                for h in range(H):
                    nc.tensor.matmul(ps[:, n0:n0 + nsz],
                                     ctx_sb[h][:, s * 128:(s + 1) * 128],
                                     wo_sb[h][:, n0:n0 + nsz],
                                     start=(h == 0), stop=False)
                nc.tensor.matmul(ps[:, n0:n0 + nsz],
                                 ones_row[0:1, 0:128],
                                 bo_sb[0:1, n0:n0 + nsz],
                                 start=False, stop=True)
            xn_t = xnpool.tile([128, D], F32, tag="xn")
            nc.sync.dma_start(xn_t[:], xn_d[s * 128:(s + 1) * 128, :])
            x = lnpool.tile([128, D], F32, tag="x")
            sacc = mpool.tile([128, 1], F32, tag="sacc")
            nc.vector.scalar_tensor_tensor(
                x[:], ps[:, 0:D], 1.0, xn_t[:],
                op0=ALU.mult, op1=ALU.add, accum_out=sacc[:])
            mu = mpool.tile([128, 1], F32, tag="mu")
            nc.vector.tensor_scalar_mul(mu[:], sacc[:], 1.0 / D)
            sq = lnpool.tile([128, D], F32, tag="sq")
            vacc = mpool.tile([128, 1], F32, tag="vacc")
            nc.vector.scalar_tensor_tensor(sq[:], x[:], mu[:], x[:],
                                           op0=ALU.subtract, op1=ALU.mult,
                                           accum_out=vacc[:])
            var_t = mpool.tile([128, 1], F32, tag="var")
            nc.vector.tensor_scalar(var_t[:], vacc[:], 1.0 / D, EPS,
                                    op0=ALU.mult, op1=ALU.add)
            lnv = mpool.tile([128, 1], F32, tag="lnv")
            nc.scalar.activation(lnv[:], var_t[:], AF.Ln)
            rstd = mpool.tile([128, 1], F32, tag="rstd")
            nc.scalar.activation(rstd[:], lnv[:], AF.Exp, scale=-0.5)
            y = lnpool.tile([128, D], F32, tag="y")
            nc.vector.tensor_scalar(y[:], x[:], mu[:], rstd[:],
                                    op0=ALU.subtract, op1=ALU.mult)
            g = lnpool.tile([128, D], F32, tag="g")
            nc.vector.scalar_tensor_tensor(g[:], y[:], 1.0, gb_sb[:],
                                           op0=ALU.mult, op1=ALU.mult)
            o = lnpool.tile([128, D], F32, tag="o")
            nc.vector.tensor_tensor(o[:], g[:], bb_sb[:], ALU.add)
            nc.sync.dma_start(out_d[s * 128:(s + 1) * 128, :], o[:])

    return _finalize(ctx, nc, key)


def prep_inputs(hidden_states, attention_mask, head_mask, Wq, bq, Wk, bk,
                Wv, bv, Wo, bo, gamma, beta):
    """Host-side shard + layout prep. Returns per-core input maps."""
    bf = ml_dtypes.bfloat16
    hidden = np.asarray(hidden_states, np.float32)
    hm = np.asarray(head_mask, np.float32)
    hm_dv = np.repeat(hm, HD)  # per dv column
    wqt = np.ascontiguousarray(np.asarray(Wq, np.float32).T).astype(bf)
    wkt = np.ascontiguousarray(np.asarray(Wk, np.float32).T).astype(bf)
    wvt = np.ascontiguousarray(np.asarray(Wv, np.float32).T * hm_dv[None, :]).astype(bf)
    wot = np.ascontiguousarray(np.asarray(Wo, np.float32).T).astype(bf)
    bqr = np.asarray(bq, np.float32).reshape(1, D).astype(bf)
    bkr = np.asarray(bk, np.float32).reshape(1, D).astype(bf)
    bvr = (np.asarray(bv, np.float32) * hm_dv).reshape(1, D).astype(bf)
    bor = np.asarray(bo, np.float32).reshape(1, D).astype(bf)
    gammab = np.ascontiguousarray(
        np.broadcast_to(np.asarray(gamma, np.float32), (128, D)))
    betab = np.ascontiguousarray(
        np.broadcast_to(np.asarray(beta, np.float32), (128, D)))
    am = np.asarray(attention_mask, np.float32).reshape(B, S)

    in_maps = []
    for b in range(B):
        maskc = np.ascontiguousarray(am[b].reshape(S_CH, 128).T)  # [128, 8]
        in_maps.append({
            "xt": np.ascontiguousarray(hidden[b].T).astype(bf),
            "xn": np.ascontiguousarray(hidden[b]),
            "wqt": wqt, "wkt": wkt, "wvt": wvt, "wot": wot,
            "bqr": bqr, "bkr": bkr, "bvr": bvr, "bor": bor,
            "maskc": maskc, "gammab": gammab, "betab": betab,
        })
    return in_maps


def run(in_maps, trace=False, trace_cores=None, phases=4):
    _ensure_path()
    from concourse.bass_utils import run_bass_kernel_spmd
    nc = build_program(phases)
    return run_bass_kernel_spmd(nc, in_maps, list(range(NCORES)), trace=trace,
                                trace_cores=trace_cores)


def kernel(**inputs) -> np.ndarray:
    in_maps = prep_inputs(**inputs)
    res = run(in_maps)
    out = np.stack([res.results[b]["out"] for b in range(B)], axis=0)
    return out.astype(np.float32)


# revision 32
# speedup vs baseline: 1.2769x; 1.2769x over previous
"""BERT attention layer (nn_BertAttention) as a Bass/Tile kernel on 8 trn2 cores.

Sharding: data-parallel over batch (B=8 -> 1 batch element per core, no
collectives). Each core computes QKV projections, per-head attention,
masked output projection, residual + LayerNorm for its batch element.

Layout strategy (per core, S=1024, D=768, H=12, HD=64):
  - hidden^T (x_t [D, S]) feeds projections; Q,K produced transposed
    (q_t/k_t [D, S], head pair per 128-partition chunk), V natural [S, D].
  - scores computed transposed: s_t[k, q] = K @ Q^T per head. ACT exp
    folds the 1/sqrt(HD) scale and the per-k attention-mask bias.
  - PV: ctx^T[d, q] with e_t as moving operand; a concurrent all-ones
    [128,32] matmul in a disjoint column group produces the softmax
    denominator rows for free.
  - denominator reciprocal: one row is bounced through DRAM into a
    [128, 8] partition spread (so the iterative-divide DVE reciprocal
    touches 8 elems/lane instead of 1024), then broadcast back to
    [64, S] via a zero-stride DRAM read for the normalization multiply.
  - out-projection accumulates all 12 heads (+bias row) into psum per
    q-chunk; residual + LayerNorm fused on DVE with ACT ln/exp for
    rsqrt (keeps a single activation table set for the whole kernel).
  - head_mask is folded into Wv/bv host-side; biases enter as augmented
    rank-1 matmuls (ones x bias-row).
"""

import numpy as np
import ml_dtypes

B, S, D = 8, 1024, 768
H, HD = 12, 64
NCORES = 8
EPS = 1e-12
DI_CH = D // 128   # 6 contraction chunks
S_CH = S // 128    # 8 sequence chunks
PAIRS = H // 2     # 6 head pairs

_CACHE: dict = {}


def _ensure_path():
    import sys
    if "/opt/trn_rl_repo" not in sys.path:
        sys.path.insert(0, "/opt/trn_rl_repo")


def _finalize(ctx, nc, key):
    """Close the TileContext (schedules), compile, cache."""
    ctx.close()
    nc.compile()
    _CACHE[key] = nc
    return nc


def build_program(phases=4):
    """Build (once) the Bass program shared by all cores."""
    key = ("nc", phases)
    if key in _CACHE:
        return _CACHE[key]
    _ensure_path()
    from contextlib import ExitStack
    import concourse.bass as bass
    import concourse.bacc as bacc
    import concourse.mybir as mybir
    import concourse.tile as tile

    F32 = mybir.dt.float32
    BF16 = mybir.dt.bfloat16
    AF = mybir.ActivationFunctionType
    ALU = mybir.AluOpType

    nc = bacc.Bacc("TRN2", target_bir_lowering=False, debug=False)

    xt_d = nc.declare_dram_parameter("xt", [D, S], BF16, isOutput=False)
    xn_d = nc.declare_dram_parameter("xn", [S, D], F32, isOutput=False)
    wqt_d = nc.declare_dram_parameter("wqt", [D, D], BF16, isOutput=False)
    wkt_d = nc.declare_dram_parameter("wkt", [D, D], BF16, isOutput=False)
    wvt_d = nc.declare_dram_parameter("wvt", [D, D], BF16, isOutput=False)
    wot_d = nc.declare_dram_parameter("wot", [D, D], BF16, isOutput=False)
    bq_d = nc.declare_dram_parameter("bqr", [1, D], BF16, isOutput=False)
    bk_d = nc.declare_dram_parameter("bkr", [1, D], BF16, isOutput=False)
    bv_d = nc.declare_dram_parameter("bvr", [1, D], BF16, isOutput=False)
    bo_d = nc.declare_dram_parameter("bor", [1, D], BF16, isOutput=False)
    mask_d = nc.declare_dram_parameter("maskc", [128, S_CH], F32, isOutput=False)
    gb_d = nc.declare_dram_parameter("gammab", [128, D], F32, isOutput=False)
    bb_d = nc.declare_dram_parameter("betab", [128, D], F32, isOutput=False)
    out_d = nc.declare_dram_parameter("out", [S, D], F32, isOutput=True)

    dn_b1 = nc.dram_tensor("dn_b1", [H, S], F32)   # denominator bounce
    dn_b2 = nc.dram_tensor("dn_b2", [H, S], F32)   # reciprocal bounce

    with ExitStack() as ctx:
        tc = ctx.enter_context(tile.TileContext(nc))
        const = ctx.enter_context(tc.tile_pool(name="const", bufs=1))
        wpool = ctx.enter_context(tc.tile_pool(name="w", bufs=1))
        apool = ctx.enter_context(tc.tile_pool(name="act", bufs=1))
        epool = ctx.enter_context(tc.tile_pool(name="e", bufs=10))
        xnpool = ctx.enter_context(tc.tile_pool(name="xn", bufs=2))
        lnpool = ctx.enter_context(tc.tile_pool(name="ln", bufs=2))
        mpool = ctx.enter_context(tc.tile_pool(name="mini", bufs=4))
        rpool = ctx.enter_context(tc.tile_pool(name="rp", bufs=2))
        psum = ctx.enter_context(tc.tile_pool(name="ps", bufs=2, space="PSUM"))

        # ---- constants / small tensors ----
        ones_row = const.tile([1, S], BF16, tag="ones_row")
        nc.vector.memset(ones_row[:], 1.0)
        mask_sb = const.tile([128, S_CH], F32, tag="mask")
        nc.sync.dma_start(mask_sb[:], mask_d[:])
        gb_sb = const.tile([128, D], F32, tag="gb")
        nc.sync.dma_start(gb_sb[:], gb_d[:])
        bb_sb = const.tile([128, D], F32, tag="bb")
        nc.sync.dma_start(bb_sb[:], bb_d[:])
        bq_sb = const.tile([1, D], BF16, tag="bq")
        nc.sync.dma_start(bq_sb[:], bq_d[:])
        bk_sb = const.tile([1, D], BF16, tag="bk")
        nc.sync.dma_start(bk_sb[:], bk_d[:])
        bv_sb = const.tile([1, D], BF16, tag="bv")
        nc.sync.dma_start(bv_sb[:], bv_d[:])
        bo_sb = const.tile([1, D], BF16, tag="bo")
        nc.sync.dma_start(bo_sb[:], bo_d[:])

        # ---- bulk input loads ----
        xt_sb = []
        for c in range(DI_CH):
            t = wpool.tile([128, S], BF16, tag=f"xt{c}")
            nc.sync.dma_start(t[:], xt_d[c * 128:(c + 1) * 128, :])
            xt_sb.append(t)
        wq_sb, wk_sb, wv_sb = [], [], []
        for name, dram, lst in (("wq", wqt_d, wq_sb), ("wk", wkt_d, wk_sb),
                                ("wv", wvt_d, wv_sb)):
            for c in range(DI_CH):
                t = wpool.tile([128, D], BF16, tag=f"{name}{c}")
                nc.sync.dma_start(t[:], dram[c * 128:(c + 1) * 128, :])
                lst.append(t)
        # Wo stored per head at partition base 0: all out-projection
        # matmuls then accumulate with a uniform contract base (mixing
        # contract bases 0/64 inside one psum accumulation group faults
        # the device).
        wo_sb = []
        for h in range(H):
            t = wpool.tile([64, D], BF16, tag=f"wo{h}")
            nc.sync.dma_start(t[:], wot_d[h * HD:(h + 1) * HD, :])
            wo_sb.append(t)

        VSPL = [(0, 512), (512, 256)]  # free-dim splits for D=768 outputs

        # ---- phase 1: V projection -> v_sb[s] [128 tokens, 768 dv] ----
        # v stored per head with an appended all-ones 65th column: the PV
        # matmul then produces the softmax denominator as psum row 64 of
        # the same accumulation group (no extra matmuls).
        v_sb = []
        for s in range(S_CH):
            vt = apool.tile([128, H * (HD + 1)], BF16, tag=f"v{s}")
            vt3 = vt.rearrange("p (h w) -> p h w", h=H)
            nc.vector.memset(vt3[:, :, HD:HD + 1], 1.0)
            ps = psum.tile([128, S], F32, tag="big")
            for n0, nsz in VSPL:
                for di in range(DI_CH):
                    nc.tensor.matmul(ps[:, n0:n0 + nsz],
                                     xt_sb[di][:, s * 128:(s + 1) * 128],
                                     wv_sb[di][:, n0:n0 + nsz],
                                     start=(di == 0), stop=False)
                nc.tensor.matmul(ps[:, n0:n0 + nsz],
                                 ones_row[0:1, 0:128],
                                 bv_sb[0:1, n0:n0 + nsz],
                                 start=False, stop=True)
            nc.vector.tensor_copy(
                vt3[:, :, 0:HD],
                ps[:, 0:D].rearrange("p (h w) -> p h w", h=H))
            v_sb.append(vt)

        if phases < 2:
            for s in range(S_CH):
                o = lnpool.tile([128, D], F32, tag="o")
                nc.vector.tensor_copy(o[:], v_sb[s][:])
                nc.sync.dma_start(out_d[s * 128:(s + 1) * 128, :], o[:])
            return _finalize(ctx, nc, key)

        # ---- phase 2: Q/K projections (transposed layout) ----
        q_sb, k_sb = [], []
        for which, w_sb, b_sb, lst in (("q", wq_sb, bq_sb, q_sb),
                                       ("k", wk_sb, bk_sb, k_sb)):
            for c in range(DI_CH):
                t = apool.tile([128, S], BF16, tag=f"{which}t{c}")
                ps = psum.tile([128, S], F32, tag="big")
                for n in range(2):
                    nsl = slice(n * 512, (n + 1) * 512)
                    for di in range(DI_CH):
                        nc.tensor.matmul(ps[:, nsl],
                                         w_sb[di][:, c * 128:(c + 1) * 128],
                                         xt_sb[di][:, nsl],
                                         start=(di == 0), stop=False)
                    nc.tensor.matmul(ps[:, nsl],
                                     b_sb[0:1, c * 128:(c + 1) * 128],
                                     ones_row[0:1, nsl],
                                     start=False, stop=True)
                nc.vector.tensor_copy(t[:], ps[:])
                lst.append(t)

        if phases < 3:
            for c in range(DI_CH):
                o = lnpool.tile([128, D], F32, tag="o2")
                nc.vector.tensor_copy(o[:], q_sb[c][:, 0:D])
                nc.sync.dma_start(out_d[c * 128:(c + 1) * 128, :], o[:])
            return _finalize(ctx, nc, key)

        # ---- phase 3: attention, head pair per chunk ----
        ctx_sb = []
        for h in range(H):
            ctx_t = apool.tile([64, S], BF16, tag=f"ctx{h}", name=f"ctx{h}")
            ctx_sb.append(ctx_t)
        for p in range(PAIRS):
            for half in range(2):  # q/k rows of the pair chunk per head
                h = 2 * p + half
                rlo, rhi = (0, 64) if half == 0 else (64, 128)
                e_tiles = []
                for c in range(S_CH):
                    ss = psum.tile([128, S], F32, tag="ss")
                    for n in range(2):
                        nsl = slice(n * 512, (n + 1) * 512)
                        nc.tensor.matmul(ss[:, nsl],
                                         k_sb[p][rlo:rhi, c * 128:(c + 1) * 128],
                                         q_sb[p][rlo:rhi, nsl],
                                         start=True, stop=True)
                    e = epool.tile([128, S], BF16, tag="e")
                    nc.scalar.activation(e[:], ss[:], AF.Exp,
                                         bias=mask_sb[:, c:c + 1], scale=0.125)
                    e_tiles.append(e)
                # PV with V-augmented ones column: out rows 0:63 = ctx^T,
                # row 64 = softmax denominator, one accumulation group.
                ct = psum.tile([128, S], F32, tag="big")
                for c in range(S_CH):
                    st, sp = (c == 0), (c == S_CH - 1)
                    for n in range(2):
                        nsl = slice(n * 512, (n + 1) * 512)
                        nc.tensor.matmul(ct[0:HD + 1, nsl],
                                         v_sb[c][:, h * (HD + 1):(h + 1) * (HD + 1)],
                                         e_tiles[c][:, nsl], start=st, stop=sp)
                # denominator -> DRAM -> [128, 8] spread -> reciprocal ->
                # DRAM -> zero-stride broadcast [64, S]
                dcp = rpool.tile([1, S], F32, tag="dcp", bufs=1)
                nc.vector.tensor_copy(dcp[:], ct[HD:HD + 1, :])
                nc.sync.dma_start(dn_b1.ap()[h:h + 1, :], dcp[:])
                rs = rpool.tile([128, S // 128], F32, tag="rs")
                nc.sync.dma_start(
                    rs[:], dn_b1.ap()[h:h + 1, :].rearrange(
                        "one (p j) -> (one p) j", p=128))
                rc = rpool.tile([128, S // 128], F32, tag="rc")
                nc.vector.reciprocal(rc[:], rs[:])
                nc.sync.dma_start(
                    dn_b2.ap()[h:h + 1, :].rearrange(
                        "one (p j) -> (one p) j", p=128), rc[:])
                bc = rpool.tile([64, S], F32, tag="bc")
                nc.sync.dma_start(bc[:], bass.AP(dn_b2, h * S, [[0, 64], [1, S]]))
                nc.vector.tensor_tensor(ctx_sb[h][:], ct[0:HD, :],
                                        bc[:], ALU.mult)

        if phases < 4:
            for h in range(H):
                o = lnpool.tile([64, S], F32, tag="octx")
                nc.vector.tensor_copy(o[:], ctx_sb[h][:])
                nc.sync.dma_start(out_d[h * 64:(h + 1) * 64, 0:D], o[:, 0:D])
            return _finalize(ctx, nc, key)

        # ---- phase 4: output projection + residual + LayerNorm ----
        for s in range(S_CH):
            ps = psum.tile([128, S], F32, tag="big")
            for n0, nsz in VSPL:
                for h in range(H):
                    nc.tensor.matmul(ps[:, n0:n0 + nsz],
                                     ctx_sb[h][:, s * 128:(s + 1) * 128],
                                     wo_sb[h][:, n0:n0 + nsz],
                                     start=(h == 0), stop=False)
                nc.tensor.matmul(ps[:, n0:n0 + nsz],
                                 ones_row[0:1, 0:128],
                                 bo_sb[0:1, n0:n0 + nsz],
                                 start=False, stop=True)
            xn_t = xnpool.tile([128, D], F32, tag="xn")
            nc.sync.dma_start(xn_t[:], xn_d[s * 128:(s + 1) * 128, :])
            x = lnpool.tile([128, D], F32, tag="x")
            sacc = mpool.tile([128, 1], F32, tag="sacc")
            nc.vector.scalar_tensor_tensor(
                x[:], ps[:, 0:D], 1.0, xn_t[:],
                op0=ALU.mult, op1=ALU.add, accum_out=sacc[:])
            mu = mpool.tile([128, 1], F32, tag="mu")
            nc.vector.tensor_scalar_mul(mu[:], sacc[:], 1.0 / D)
            sq = lnpool.tile([128, D], F32, tag="sq")
            vacc = mpool.tile([128, 1], F32, tag="vacc")
            nc.vector.scalar_tensor_tensor(sq[:], x[:], mu[:], x[:],
                                           op0=ALU.subtract, op1=ALU.mult,
                                           accum_out=vacc[:])
            var_t = mpool.tile([128, 1], F32, tag="var")
            nc.vector.tensor_scalar(var_t[:], vacc[:], 1.0 / D, EPS,
                                    op0=ALU.mult, op1=ALU.add)
            lnv = mpool.tile([128, 1], F32, tag="lnv")
            nc.scalar.activation(lnv[:], var_t[:], AF.Ln)
            rstd = mpool.tile([128, 1], F32, tag="rstd")
            nc.scalar.activation(rstd[:], lnv[:], AF.Exp, scale=-0.5)
            y = lnpool.tile([128, D], F32, tag="y")
            nc.vector.tensor_scalar(y[:], x[:], mu[:], rstd[:],
                                    op0=ALU.subtract, op1=ALU.mult)
            g = lnpool.tile([128, D], F32, tag="g")
            nc.vector.scalar_tensor_tensor(g[:], y[:], 1.0, gb_sb[:],
                                           op0=ALU.mult, op1=ALU.mult)
            o = lnpool.tile([128, D], F32, tag="o")
            nc.vector.tensor_tensor(o[:], g[:], bb_sb[:], ALU.add)
            nc.sync.dma_start(out_d[s * 128:(s + 1) * 128, :], o[:])

    return _finalize(ctx, nc, key)


def prep_inputs(hidden_states, attention_mask, head_mask, Wq, bq, Wk, bk,
                Wv, bv, Wo, bo, gamma, beta):
    """Host-side shard + layout prep. Returns per-core input maps."""
    bf = ml_dtypes.bfloat16
    hidden = np.asarray(hidden_states, np.float32)
    hm = np.asarray(head_mask, np.float32)
    hm_dv = np.repeat(hm, HD)  # per dv column
    wqt = np.ascontiguousarray(np.asarray(Wq, np.float32).T).astype(bf)
    wkt = np.ascontiguousarray(np.asarray(Wk, np.float32).T).astype(bf)
    wvt = np.ascontiguousarray(np.asarray(Wv, np.float32).T * hm_dv[None, :]).astype(bf)
    wot = np.ascontiguousarray(np.asarray(Wo, np.float32).T).astype(bf)
    bqr = np.asarray(bq, np.float32).reshape(1, D).astype(bf)
    bkr = np.asarray(bk, np.float32).reshape(1, D).astype(bf)
    bvr = (np.asarray(bv, np.float32) * hm_dv).reshape(1, D).astype(bf)
    bor = np.asarray(bo, np.float32).reshape(1, D).astype(bf)
    gammab = np.ascontiguousarray(
        np.broadcast_to(np.asarray(gamma, np.float32), (128, D)))
    betab = np.ascontiguousarray(
        np.broadcast_to(np.asarray(beta, np.float32), (128, D)))
    am = np.asarray(attention_mask, np.float32).reshape(B, S)

    in_maps = []
    for b in range(B):
        maskc = np.ascontiguousarray(am[b].reshape(S_CH, 128).T)  # [128, 8]
        in_maps.append({
            "xt": np.ascontiguousarray(hidden[b].T).astype(bf),
            "xn": np.ascontiguousarray(hidden[b]),
            "wqt": wqt, "wkt": wkt, "wvt": wvt, "wot": wot,
            "bqr": bqr, "bkr": bkr, "bvr": bvr, "bor": bor,
            "maskc": maskc, "gammab": gammab, "betab": betab,
        })
    return in_maps


def run(in_maps, trace=False, trace_cores=None, phases=4):
    _ensure_path()
    from concourse.bass_utils import run_bass_kernel_spmd
    nc = build_program(phases)
    return run_bass_kernel_spmd(nc, in_maps, list(range(NCORES)), trace=trace,
                                trace_cores=trace_cores)


def kernel(**inputs) -> np.ndarray:
    in_maps = prep_inputs(**inputs)
    res = run(in_maps)
    out = np.stack([res.results[b]["out"] for b in range(B)], axis=0)
    return out.astype(np.float32)


# revision 33
# speedup vs baseline: 1.3090x; 1.0251x over previous
"""BERT attention layer (nn_BertAttention) as a Bass/Tile kernel on 8 trn2 cores.

Sharding: data-parallel over batch (B=8 -> 1 batch element per core, no
collectives). Each core computes QKV projections, per-head attention,
masked output projection, residual + LayerNorm for its batch element.

Layout strategy (per core, S=1024, D=768, H=12, HD=64):
  - hidden^T (x_t [D, S]) feeds projections; Q,K produced transposed
    (q_t/k_t [D, S], head pair per 128-partition chunk), V natural [S, D].
  - scores computed transposed: s_t[k, q] = K @ Q^T per head. ACT exp
    folds the 1/sqrt(HD) scale and the per-k attention-mask bias.
  - PV: ctx^T[d, q] with e_t as moving operand; a concurrent all-ones
    [128,32] matmul in a disjoint column group produces the softmax
    denominator rows for free.
  - denominator reciprocal: one row is bounced through DRAM into a
    [128, 8] partition spread (so the iterative-divide DVE reciprocal
    touches 8 elems/lane instead of 1024), then broadcast back to
    [64, S] via a zero-stride DRAM read for the normalization multiply.
  - out-projection accumulates all 12 heads (+bias row) into psum per
    q-chunk; residual + LayerNorm fused on DVE with ACT ln/exp for
    rsqrt (keeps a single activation table set for the whole kernel).
  - head_mask is folded into Wv/bv host-side; biases enter as augmented
    rank-1 matmuls (ones x bias-row).
"""

import numpy as np
import ml_dtypes

B, S, D = 8, 1024, 768
H, HD = 12, 64
NCORES = 8
EPS = 1e-12
DI_CH = D // 128   # 6 contraction chunks
S_CH = S // 128    # 8 sequence chunks
PAIRS = H // 2     # 6 head pairs

_CACHE: dict = {}


def _ensure_path():
    import sys
    if "/opt/trn_rl_repo" not in sys.path:
        sys.path.insert(0, "/opt/trn_rl_repo")


def _finalize(ctx, nc, key):
    """Close the TileContext (schedules), compile, cache."""
    ctx.close()
    nc.compile()
    _CACHE[key] = nc
    return nc


def build_program(phases=4):
    """Build (once) the Bass program shared by all cores."""
    key = ("nc", phases)
    if key in _CACHE:
        return _CACHE[key]
    _ensure_path()
    from contextlib import ExitStack
    import concourse.bass as bass
    import concourse.bacc as bacc
    import concourse.mybir as mybir
    import concourse.tile as tile

    F32 = mybir.dt.float32
    BF16 = mybir.dt.bfloat16
    AF = mybir.ActivationFunctionType
    ALU = mybir.AluOpType

    nc = bacc.Bacc("TRN2", target_bir_lowering=False, debug=False)

    xt_d = nc.declare_dram_parameter("xt", [D, S], BF16, isOutput=False)
    xn_d = nc.declare_dram_parameter("xn", [S, D], F32, isOutput=False)
    wqt_d = nc.declare_dram_parameter("wqt", [D, D], BF16, isOutput=False)
    wkt_d = nc.declare_dram_parameter("wkt", [D, D], BF16, isOutput=False)
    wvt_d = nc.declare_dram_parameter("wvt", [D, D], BF16, isOutput=False)
    wot_d = nc.declare_dram_parameter("wot", [D, D], BF16, isOutput=False)
    bq_d = nc.declare_dram_parameter("bqr", [1, D], BF16, isOutput=False)
    bk_d = nc.declare_dram_parameter("bkr", [1, D], BF16, isOutput=False)
    bv_d = nc.declare_dram_parameter("bvr", [1, D], BF16, isOutput=False)
    bo_d = nc.declare_dram_parameter("bor", [1, D], BF16, isOutput=False)
    mask_d = nc.declare_dram_parameter("maskc", [128, S_CH], F32, isOutput=False)
    gb_d = nc.declare_dram_parameter("gammab", [128, D], F32, isOutput=False)
    bb_d = nc.declare_dram_parameter("betab", [128, D], F32, isOutput=False)
    out_d = nc.declare_dram_parameter("out", [S, D], F32, isOutput=True)

    dn_b1 = nc.dram_tensor("dn_b1", [H, S], F32)   # denominator bounce
    dn_b2 = nc.dram_tensor("dn_b2", [H, S], F32)   # reciprocal bounce

    with ExitStack() as ctx:
        tc = ctx.enter_context(tile.TileContext(nc))
        const = ctx.enter_context(tc.tile_pool(name="const", bufs=1))
        wpool = ctx.enter_context(tc.tile_pool(name="w", bufs=1))
        apool = ctx.enter_context(tc.tile_pool(name="act", bufs=1))
        epool = ctx.enter_context(tc.tile_pool(name="e", bufs=10))
        xnpool = ctx.enter_context(tc.tile_pool(name="xn", bufs=2))
        lnpool = ctx.enter_context(tc.tile_pool(name="ln", bufs=2))
        mpool = ctx.enter_context(tc.tile_pool(name="mini", bufs=4))
        rpool = ctx.enter_context(tc.tile_pool(name="rp", bufs=2))
        psum = ctx.enter_context(tc.tile_pool(name="ps", bufs=2, space="PSUM"))

        # ---- constants / small tensors ----
        ones_row = const.tile([1, S], BF16, tag="ones_row")
        nc.vector.memset(ones_row[:], 1.0)
        mask_sb = const.tile([128, S_CH], F32, tag="mask")
        nc.sync.dma_start(mask_sb[:], mask_d[:])
        gb_sb = const.tile([128, D], F32, tag="gb")
        nc.sync.dma_start(gb_sb[:], gb_d[:])
        bb_sb = const.tile([128, D], F32, tag="bb")
        nc.sync.dma_start(bb_sb[:], bb_d[:])
        bq_sb = const.tile([1, D], BF16, tag="bq")
        nc.sync.dma_start(bq_sb[:], bq_d[:])
        bk_sb = const.tile([1, D], BF16, tag="bk")
        nc.sync.dma_start(bk_sb[:], bk_d[:])
        bv_sb = const.tile([1, D], BF16, tag="bv")
        nc.sync.dma_start(bv_sb[:], bv_d[:])
        bo_sb = const.tile([1, D], BF16, tag="bo")
        nc.sync.dma_start(bo_sb[:], bo_d[:])

        # ---- bulk input loads ----
        xt_sb = []
        for c in range(DI_CH):
            t = wpool.tile([128, S], BF16, tag=f"xt{c}")
            nc.sync.dma_start(t[:], xt_d[c * 128:(c + 1) * 128, :])
            xt_sb.append(t)
        wq_sb, wk_sb, wv_sb = [], [], []
        for name, dram, lst in (("wq", wqt_d, wq_sb), ("wk", wkt_d, wk_sb),
                                ("wv", wvt_d, wv_sb)):
            for c in range(DI_CH):
                t = wpool.tile([128, D], BF16, tag=f"{name}{c}")
                nc.sync.dma_start(t[:], dram[c * 128:(c + 1) * 128, :])
                lst.append(t)
        # Wo stored per head at partition base 0: all out-projection
        # matmuls then accumulate with a uniform contract base (mixing
        # contract bases 0/64 inside one psum accumulation group faults
        # the device).
        wo_sb = []
        for h in range(H):
            t = wpool.tile([64, D], BF16, tag=f"wo{h}")
            nc.sync.dma_start(t[:], wot_d[h * HD:(h + 1) * HD, :])
            wo_sb.append(t)

        VSPL = [(0, 512), (512, 256)]  # free-dim splits for D=768 outputs

        # ---- phase 1: V projection -> v_sb[s] [128 tokens, 768 dv] ----
        # v stored per head with an appended all-ones 65th column: the PV
        # matmul then produces the softmax denominator as psum row 64 of
        # the same accumulation group (no extra matmuls).
        v_sb = []
        for s in range(S_CH):
            vt = apool.tile([128, H * (HD + 1)], BF16, tag=f"v{s}")
            vt3 = vt.rearrange("p (h w) -> p h w", h=H)
            nc.vector.memset(vt3[:, :, HD:HD + 1], 1.0)
            ps = psum.tile([128, S], F32, tag="big")
            for n0, nsz in VSPL:
                for di in range(DI_CH):
                    nc.tensor.matmul(ps[:, n0:n0 + nsz],
                                     xt_sb[di][:, s * 128:(s + 1) * 128],
                                     wv_sb[di][:, n0:n0 + nsz],
                                     start=(di == 0), stop=False)
                nc.tensor.matmul(ps[:, n0:n0 + nsz],
                                 ones_row[0:1, 0:128],
                                 bv_sb[0:1, n0:n0 + nsz],
                                 start=False, stop=True)
            nc.vector.tensor_copy(
                vt3[:, :, 0:HD],
                ps[:, 0:D].rearrange("p (h w) -> p h w", h=H))
            v_sb.append(vt)

        if phases < 2:
            for s in range(S_CH):
                o = lnpool.tile([128, D], F32, tag="o")
                nc.vector.tensor_copy(o[:], v_sb[s][:])
                nc.sync.dma_start(out_d[s * 128:(s + 1) * 128, :], o[:])
            return _finalize(ctx, nc, key)

        # ---- phase 2: Q/K projections (transposed layout) ----
        q_sb, k_sb = [], []
        for which, w_sb, b_sb, lst in (("q", wq_sb, bq_sb, q_sb),
                                       ("k", wk_sb, bk_sb, k_sb)):
            for c in range(DI_CH):
                t = apool.tile([128, S], BF16, tag=f"{which}t{c}")
                ps = psum.tile([128, S], F32, tag="big")
                for n in range(2):
                    nsl = slice(n * 512, (n + 1) * 512)
                    for di in range(DI_CH):
                        nc.tensor.matmul(ps[:, nsl],
                                         w_sb[di][:, c * 128:(c + 1) * 128],
                                         xt_sb[di][:, nsl],
                                         start=(di == 0), stop=False)
                    nc.tensor.matmul(ps[:, nsl],
                                     b_sb[0:1, c * 128:(c + 1) * 128],
                                     ones_row[0:1, nsl],
                                     start=False, stop=True)
                nc.vector.tensor_copy(t[:], ps[:])
                lst.append(t)

        if phases < 3:
            for c in range(DI_CH):
                o = lnpool.tile([128, D], F32, tag="o2")
                nc.vector.tensor_copy(o[:], q_sb[c][:, 0:D])
                nc.sync.dma_start(out_d[c * 128:(c + 1) * 128, :], o[:])
            return _finalize(ctx, nc, key)

        # ---- phase 3: attention, head pair per chunk ----
        ctx_sb = []
        for h in range(H):
            ctx_t = apool.tile([64, S], BF16, tag=f"ctx{h}", name=f"ctx{h}")
            ctx_sb.append(ctx_t)
        for p in range(PAIRS):
            for half in range(2):  # q/k rows of the pair chunk per head
                h = 2 * p + half
                rlo, rhi = (0, 64) if half == 0 else (64, 128)
                e_tiles = []
                for c in range(S_CH):
                    ss = psum.tile([128, S], F32, tag="ss")
                    for n in range(2):
                        nsl = slice(n * 512, (n + 1) * 512)
                        nc.tensor.matmul(ss[:, nsl],
                                         k_sb[p][rlo:rhi, c * 128:(c + 1) * 128],
                                         q_sb[p][rlo:rhi, nsl],
                                         start=True, stop=True)
                    e = epool.tile([128, S], BF16, tag="e")
                    nc.scalar.activation(e[:], ss[:], AF.Exp,
                                         bias=mask_sb[:, c:c + 1], scale=0.125)
                    e_tiles.append(e)
                # PV with V-augmented ones column: out rows 0:63 = ctx^T,
                # row 64 = softmax denominator, one accumulation group.
                ct = psum.tile([128, S], F32, tag="big")
                for c in range(S_CH):
                    st, sp = (c == 0), (c == S_CH - 1)
                    for n in range(2):
                        nsl = slice(n * 512, (n + 1) * 512)
                        nc.tensor.matmul(ct[0:HD + 1, nsl],
                                         v_sb[c][:, h * (HD + 1):(h + 1) * (HD + 1)],
                                         e_tiles[c][:, nsl], start=st, stop=sp)
                # denominator -> DRAM -> [128, 8] spread -> reciprocal ->
                # DRAM -> zero-stride broadcast [64, S]
                dcp = rpool.tile([1, S], F32, tag="dcp", bufs=1)
                nc.vector.tensor_copy(dcp[:], ct[HD:HD + 1, :])
                nc.sync.dma_start(dn_b1.ap()[h:h + 1, :], dcp[:])
                rs = rpool.tile([128, S // 128], F32, tag="rs")
                nc.sync.dma_start(
                    rs[:], dn_b1.ap()[h:h + 1, :].rearrange(
                        "one (p j) -> (one p) j", p=128))
                rc = rpool.tile([128, S // 128], F32, tag="rc")
                nc.vector.reciprocal(rc[:], rs[:])
                nc.sync.dma_start(
                    dn_b2.ap()[h:h + 1, :].rearrange(
                        "one (p j) -> (one p) j", p=128), rc[:])
                bc = rpool.tile([64, S], F32, tag="bc")
                nc.sync.dma_start(bc[:], bass.AP(dn_b2, h * S, [[0, 64], [1, S]]))
                nc.vector.tensor_tensor(ctx_sb[h][:], ct[0:HD, :],
                                        bc[:], ALU.mult)

        if phases < 4:
            for h in range(H):
                o = lnpool.tile([64, S], F32, tag="octx")
                nc.vector.tensor_copy(o[:], ctx_sb[h][:])
                nc.sync.dma_start(out_d[h * 64:(h + 1) * 64, 0:D], o[:, 0:D])
            return _finalize(ctx, nc, key)

        # ---- phase 4: output projection + residual + LayerNorm ----
        for s in range(S_CH):
            ps = psum.tile([128, S], F32, tag="big")
            for n0, nsz in VSPL:
                for h in range(H):
                    nc.tensor.matmul(ps[:, n0:n0 + nsz],
                                     ctx_sb[h][:, s * 128:(s + 1) * 128],
                                     wo_sb[h][:, n0:n0 + nsz],
                                     start=(h == 0), stop=False)
                nc.tensor.matmul(ps[:, n0:n0 + nsz],
                                 ones_row[0:1, 0:128],
                                 bo_sb[0:1, n0:n0 + nsz],
                                 start=False, stop=True)
            xn_t = xnpool.tile([128, D], F32, tag="xn")
            nc.sync.dma_start(xn_t[:], xn_d[s * 128:(s + 1) * 128, :])
            x = lnpool.tile([128, D], F32, tag="x")
            sacc = mpool.tile([128, 1], F32, tag="sacc")
            nc.vector.scalar_tensor_tensor(
                x[:], ps[:, 0:D], 1.0, xn_t[:],
                op0=ALU.mult, op1=ALU.add, accum_out=sacc[:])
            mu = mpool.tile([128, 1], F32, tag="mu")
            nc.vector.tensor_scalar_mul(mu[:], sacc[:], 1.0 / D)
            sq = lnpool.tile([128, D], F32, tag="sq")
            vacc = mpool.tile([128, 1], F32, tag="vacc")
            nc.vector.scalar_tensor_tensor(sq[:], x[:], mu[:], x[:],
                                           op0=ALU.subtract, op1=ALU.mult,
                                           accum_out=vacc[:])
            var_t = mpool.tile([128, 1], F32, tag="var")
            nc.vector.tensor_scalar(var_t[:], vacc[:], 1.0 / D, EPS,
                                    op0=ALU.mult, op1=ALU.add)
            sd = mpool.tile([128, 1], F32, tag="sd")
            nc.scalar.activation(sd[:], var_t[:], AF.Sqrt)
            rstd = mpool.tile([128, 1], F32, tag="rstd")
            nc.vector.reciprocal(rstd[:], sd[:])
            y = lnpool.tile([128, D], F32, tag="y")
            nc.vector.tensor_scalar(y[:], x[:], mu[:], rstd[:],
                                    op0=ALU.subtract, op1=ALU.mult)
            g = lnpool.tile([128, D], F32, tag="g")
            nc.vector.scalar_tensor_tensor(g[:], y[:], 1.0, gb_sb[:],
                                           op0=ALU.mult, op1=ALU.mult)
            o = lnpool.tile([128, D], F32, tag="o")
            nc.vector.tensor_tensor(o[:], g[:], bb_sb[:], ALU.add)
            nc.sync.dma_start(out_d[s * 128:(s + 1) * 128, :], o[:])

    return _finalize(ctx, nc, key)


def prep_inputs(hidden_states, attention_mask, head_mask, Wq, bq, Wk, bk,
                Wv, bv, Wo, bo, gamma, beta):
    """Host-side shard + layout prep. Returns per-core input maps."""
    bf = ml_dtypes.bfloat16
    hidden = np.asarray(hidden_states, np.float32)
    hm = np.asarray(head_mask, np.float32)
    hm_dv = np.repeat(hm, HD)  # per dv column
    wqt = np.ascontiguousarray(np.asarray(Wq, np.float32).T).astype(bf)
    wkt = np.ascontiguousarray(np.asarray(Wk, np.float32).T).astype(bf)
    wvt = np.ascontiguousarray(np.asarray(Wv, np.float32).T * hm_dv[None, :]).astype(bf)
    wot = np.ascontiguousarray(np.asarray(Wo, np.float32).T).astype(bf)
    bqr = np.asarray(bq, np.float32).reshape(1, D).astype(bf)
    bkr = np.asarray(bk, np.float32).reshape(1, D).astype(bf)
    bvr = (np.asarray(bv, np.float32) * hm_dv).reshape(1, D).astype(bf)
    bor = np.asarray(bo, np.float32).reshape(1, D).astype(bf)
    gammab = np.ascontiguousarray(
        np.broadcast_to(np.asarray(gamma, np.float32), (128, D)))
    betab = np.ascontiguousarray(
        np.broadcast_to(np.asarray(beta, np.float32), (128, D)))
    am = np.asarray(attention_mask, np.float32).reshape(B, S)

    in_maps = []
    for b in range(B):
        maskc = np.ascontiguousarray(am[b].reshape(S_CH, 128).T)  # [128, 8]
        in_maps.append({
            "xt": np.ascontiguousarray(hidden[b].T).astype(bf),
            "xn": np.ascontiguousarray(hidden[b]),
            "wqt": wqt, "wkt": wkt, "wvt": wvt, "wot": wot,
            "bqr": bqr, "bkr": bkr, "bvr": bvr, "bor": bor,
            "maskc": maskc, "gammab": gammab, "betab": betab,
        })
    return in_maps


def run(in_maps, trace=False, trace_cores=None, phases=4):
    _ensure_path()
    from concourse.bass_utils import run_bass_kernel_spmd
    nc = build_program(phases)
    return run_bass_kernel_spmd(nc, in_maps, list(range(NCORES)), trace=trace,
                                trace_cores=trace_cores)


def kernel(**inputs) -> np.ndarray:
    in_maps = prep_inputs(**inputs)
    res = run(in_maps)
    out = np.stack([res.results[b]["out"] for b in range(B)], axis=0)
    return out.astype(np.float32)


# revision 35
# speedup vs baseline: 1.3132x; 1.0032x over previous
"""BERT attention layer (nn_BertAttention) as a Bass/Tile kernel on 8 trn2 cores.

Sharding: data-parallel over batch (B=8 -> 1 batch element per core, no
collectives). Each core computes QKV projections, per-head attention,
masked output projection, residual + LayerNorm for its batch element.

Layout strategy (per core, S=1024, D=768, H=12, HD=64):
  - hidden^T (x_t [D, S]) feeds projections; Q,K produced transposed
    (q_t/k_t [D, S], head pair per 128-partition chunk), V natural [S, D].
  - scores computed transposed: s_t[k, q] = K @ Q^T per head. ACT exp
    folds the 1/sqrt(HD) scale and the per-k attention-mask bias.
  - PV: ctx^T[d, q] with e_t as moving operand; a concurrent all-ones
    [128,32] matmul in a disjoint column group produces the softmax
    denominator rows for free.
  - denominator reciprocal: one row is bounced through DRAM into a
    [128, 8] partition spread (so the iterative-divide DVE reciprocal
    touches 8 elems/lane instead of 1024), then broadcast back to
    [64, S] via a zero-stride DRAM read for the normalization multiply.
  - out-projection accumulates all 12 heads (+bias row) into psum per
    q-chunk; residual + LayerNorm fused on DVE with ACT ln/exp for
    rsqrt (keeps a single activation table set for the whole kernel).
  - head_mask is folded into Wv/bv host-side; biases enter as augmented
    rank-1 matmuls (ones x bias-row).
"""

import numpy as np
import ml_dtypes

B, S, D = 8, 1024, 768
H, HD = 12, 64
NCORES = 8
EPS = 1e-12
DI_CH = D // 128   # 6 contraction chunks
S_CH = S // 128    # 8 sequence chunks
PAIRS = H // 2     # 6 head pairs

_CACHE: dict = {}


def _ensure_path():
    import sys
    if "/opt/trn_rl_repo" not in sys.path:
        sys.path.insert(0, "/opt/trn_rl_repo")


def _finalize(ctx, nc, key):
    """Close the TileContext (schedules), compile, cache."""
    ctx.close()
    nc.compile()
    _CACHE[key] = nc
    return nc


def build_program(phases=4):
    """Build (once) the Bass program shared by all cores."""
    key = ("nc", phases)
    if key in _CACHE:
        return _CACHE[key]
    _ensure_path()
    from contextlib import ExitStack
    import concourse.bass as bass
    import concourse.bacc as bacc
    import concourse.mybir as mybir
    import concourse.tile as tile

    F32 = mybir.dt.float32
    BF16 = mybir.dt.bfloat16
    AF = mybir.ActivationFunctionType
    ALU = mybir.AluOpType

    nc = bacc.Bacc("TRN2", target_bir_lowering=False, debug=False)

    xt_d = nc.declare_dram_parameter("xt", [D, S], BF16, isOutput=False)
    xn_d = nc.declare_dram_parameter("xn", [S, D], F32, isOutput=False)
    wqt_d = nc.declare_dram_parameter("wqt", [D, D], BF16, isOutput=False)
    wkt_d = nc.declare_dram_parameter("wkt", [D, D], BF16, isOutput=False)
    wvt_d = nc.declare_dram_parameter("wvt", [D, D], BF16, isOutput=False)
    wot_d = nc.declare_dram_parameter("wot", [D, D], BF16, isOutput=False)
    bq_d = nc.declare_dram_parameter("bqr", [1, D], BF16, isOutput=False)
    bk_d = nc.declare_dram_parameter("bkr", [1, D], BF16, isOutput=False)
    bv_d = nc.declare_dram_parameter("bvr", [1, D], BF16, isOutput=False)
    bo_d = nc.declare_dram_parameter("bor", [1, D], BF16, isOutput=False)
    mask_d = nc.declare_dram_parameter("maskc", [128, S_CH], F32, isOutput=False)
    gb_d = nc.declare_dram_parameter("gammab", [128, D], F32, isOutput=False)
    bb_d = nc.declare_dram_parameter("betab", [128, D], F32, isOutput=False)
    out_d = nc.declare_dram_parameter("out", [S, D], F32, isOutput=True)

    dn_b1 = nc.dram_tensor("dn_b1", [H, S], F32)   # denominator bounce
    dn_b2 = nc.dram_tensor("dn_b2", [H, S], F32)   # reciprocal bounce

    with ExitStack() as ctx:
        tc = ctx.enter_context(tile.TileContext(nc))
        const = ctx.enter_context(tc.tile_pool(name="const", bufs=1))
        wpool = ctx.enter_context(tc.tile_pool(name="w", bufs=1))
        apool = ctx.enter_context(tc.tile_pool(name="act", bufs=1))
        epool = ctx.enter_context(tc.tile_pool(name="e", bufs=12))
        xnpool = ctx.enter_context(tc.tile_pool(name="xn", bufs=2))
        lnpool = ctx.enter_context(tc.tile_pool(name="ln", bufs=2))
        mpool = ctx.enter_context(tc.tile_pool(name="mini", bufs=4))
        rpool = ctx.enter_context(tc.tile_pool(name="rp", bufs=2))
        psum = ctx.enter_context(tc.tile_pool(name="ps", bufs=2, space="PSUM"))

        # ---- constants / small tensors ----
        ones_row = const.tile([1, S], BF16, tag="ones_row")
        nc.vector.memset(ones_row[:], 1.0)
        mask_sb = const.tile([128, S_CH], F32, tag="mask")
        nc.sync.dma_start(mask_sb[:], mask_d[:])
        gb_sb = const.tile([128, D], F32, tag="gb")
        nc.sync.dma_start(gb_sb[:], gb_d[:])
        bb_sb = const.tile([128, D], F32, tag="bb")
        nc.sync.dma_start(bb_sb[:], bb_d[:])
        bq_sb = const.tile([1, D], BF16, tag="bq")
        nc.sync.dma_start(bq_sb[:], bq_d[:])
        bk_sb = const.tile([1, D], BF16, tag="bk")
        nc.sync.dma_start(bk_sb[:], bk_d[:])
        bv_sb = const.tile([1, D], BF16, tag="bv")
        nc.sync.dma_start(bv_sb[:], bv_d[:])
        bo_sb = const.tile([1, D], BF16, tag="bo")
        nc.sync.dma_start(bo_sb[:], bo_d[:])

        # ---- bulk input loads ----
        xt_sb = []
        for c in range(DI_CH):
            t = wpool.tile([128, S], BF16, tag=f"xt{c}")
            nc.sync.dma_start(t[:], xt_d[c * 128:(c + 1) * 128, :])
            xt_sb.append(t)
        wq_sb, wk_sb, wv_sb = [], [], []
        for name, dram, lst in (("wq", wqt_d, wq_sb), ("wk", wkt_d, wk_sb),
                                ("wv", wvt_d, wv_sb)):
            for c in range(DI_CH):
                t = wpool.tile([128, D], BF16, tag=f"{name}{c}")
                nc.sync.dma_start(t[:], dram[c * 128:(c + 1) * 128, :])
                lst.append(t)
        # Wo stored per head at partition base 0: all out-projection
        # matmuls then accumulate with a uniform contract base (mixing
        # contract bases 0/64 inside one psum accumulation group faults
        # the device).
        wo_sb = []
        for h in range(H):
            t = wpool.tile([64, D], BF16, tag=f"wo{h}")
            nc.sync.dma_start(t[:], wot_d[h * HD:(h + 1) * HD, :])
            wo_sb.append(t)

        VSPL = [(0, 512), (512, 256)]  # free-dim splits for D=768 outputs

        # ---- phase 1: V projection -> v_sb[s] [128 tokens, 768 dv] ----
        # v stored per head with an appended all-ones 65th column: the PV
        # matmul then produces the softmax denominator as psum row 64 of
        # the same accumulation group (no extra matmuls).
        v_sb = []
        for s in range(S_CH):
            vt = apool.tile([128, H * (HD + 1)], BF16, tag=f"v{s}")
            vt3 = vt.rearrange("p (h w) -> p h w", h=H)
            nc.vector.memset(vt3[:, :, HD:HD + 1], 1.0)
            ps = psum.tile([128, S], F32, tag="big")
            for n0, nsz in VSPL:
                for di in range(DI_CH):
                    nc.tensor.matmul(ps[:, n0:n0 + nsz],
                                     xt_sb[di][:, s * 128:(s + 1) * 128],
                                     wv_sb[di][:, n0:n0 + nsz],
                                     start=(di == 0), stop=False)
                nc.tensor.matmul(ps[:, n0:n0 + nsz],
                                 ones_row[0:1, 0:128],
                                 bv_sb[0:1, n0:n0 + nsz],
                                 start=False, stop=True)
            nc.vector.tensor_copy(
                vt3[:, :, 0:HD],
                ps[:, 0:D].rearrange("p (h w) -> p h w", h=H))
            v_sb.append(vt)

        if phases < 2:
            for s in range(S_CH):
                o = lnpool.tile([128, D], F32, tag="o")
                nc.vector.tensor_copy(o[:], v_sb[s][:])
                nc.sync.dma_start(out_d[s * 128:(s + 1) * 128, :], o[:])
            return _finalize(ctx, nc, key)

        # ---- phase 2: Q/K projections (transposed layout) ----
        q_sb, k_sb = [], []
        for which, w_sb, b_sb, lst in (("q", wq_sb, bq_sb, q_sb),
                                       ("k", wk_sb, bk_sb, k_sb)):
            for c in range(DI_CH):
                t = apool.tile([128, S], BF16, tag=f"{which}t{c}")
                ps = psum.tile([128, S], F32, tag="big")
                for n in range(2):
                    nsl = slice(n * 512, (n + 1) * 512)
                    for di in range(DI_CH):
                        nc.tensor.matmul(ps[:, nsl],
                                         w_sb[di][:, c * 128:(c + 1) * 128],
                                         xt_sb[di][:, nsl],
                                         start=(di == 0), stop=False)
                    nc.tensor.matmul(ps[:, nsl],
                                     b_sb[0:1, c * 128:(c + 1) * 128],
                                     ones_row[0:1, nsl],
                                     start=False, stop=True)
                nc.vector.tensor_copy(t[:], ps[:])
                lst.append(t)

        if phases < 3:
            for c in range(DI_CH):
                o = lnpool.tile([128, D], F32, tag="o2")
                nc.vector.tensor_copy(o[:], q_sb[c][:, 0:D])
                nc.sync.dma_start(out_d[c * 128:(c + 1) * 128, :], o[:])
            return _finalize(ctx, nc, key)

        # ---- phase 3: attention, head pair per chunk ----
        ctx_sb = []
        for h in range(H):
            ctx_t = apool.tile([64, S], BF16, tag=f"ctx{h}", name=f"ctx{h}")
            ctx_sb.append(ctx_t)
        # Two-stage software pipeline over heads: head h's scores/exp are
        # emitted interleaved with head h-1's PV matmuls, so the PE stream
        # always has exp-independent work and never idles (keeps HAM warm).
        e_by_head = {}

        def emit_scores(h):
            p, half = h // 2, h % 2
            rlo, rhi = (0, 64) if half == 0 else (64, 128)
            ss = psum.tile([128, S], F32, tag="ss")
            c = emit_scores.c
            for n in range(2):
                nsl = slice(n * 512, (n + 1) * 512)
                nc.tensor.matmul(ss[:, nsl],
                                 k_sb[p][rlo:rhi, c * 128:(c + 1) * 128],
                                 q_sb[p][rlo:rhi, nsl],
                                 start=True, stop=True)
            e = epool.tile([128, S], BF16, tag="e")
            nc.scalar.activation(e[:], ss[:], AF.Exp,
                                 bias=mask_sb[:, c:c + 1], scale=0.125)
            e_by_head[h].append(e)

        def emit_pv(h, ct, c):
            st, sp = (c == 0), (c == S_CH - 1)
            for n in range(2):
                nsl = slice(n * 512, (n + 1) * 512)
                nc.tensor.matmul(ct[0:HD + 1, nsl],
                                 v_sb[c][:, h * (HD + 1):(h + 1) * (HD + 1)],
                                 e_by_head[h][c][:, nsl], start=st, stop=sp)

        def emit_norm(h, ct):
            # denominator -> DRAM -> [128, 8] spread -> reciprocal ->
            # DRAM -> zero-stride broadcast [64, S] -> normalize
            dcp = rpool.tile([1, S], F32, tag="dcp", bufs=1)
            nc.vector.tensor_copy(dcp[:], ct[HD:HD + 1, :])
            nc.sync.dma_start(dn_b1.ap()[h:h + 1, :], dcp[:])
            rs = rpool.tile([128, S // 128], F32, tag="rs")
            nc.sync.dma_start(
                rs[:], dn_b1.ap()[h:h + 1, :].rearrange(
                    "one (p j) -> (one p) j", p=128))
            rc = rpool.tile([128, S // 128], F32, tag="rc")
            nc.vector.reciprocal(rc[:], rs[:])
            nc.sync.dma_start(
                dn_b2.ap()[h:h + 1, :].rearrange(
                    "one (p j) -> (one p) j", p=128), rc[:])
            bc = rpool.tile([64, S], F32, tag="bc", bufs=1)
            nc.sync.dma_start(bc[:], bass.AP(dn_b2, h * S, [[0, 64], [1, S]]))
            nc.vector.tensor_tensor(ctx_sb[h][:], ct[0:HD, :],
                                    bc[:], ALU.mult)

        ct_prev = None
        for h in range(H):
            e_by_head[h] = []
            for c in range(S_CH):
                emit_scores.c = c
                emit_scores(h)
                if h > 0:
                    emit_pv(h - 1, ct_prev, c)
            if h > 0:
                emit_norm(h - 1, ct_prev)
                e_by_head.pop(h - 1)
            ct_prev = psum.tile([128, S], F32, tag="big", name="ct")
        for c in range(S_CH):
            emit_pv(H - 1, ct_prev, c)
        emit_norm(H - 1, ct_prev)

        if phases < 4:
            for h in range(H):
                o = lnpool.tile([64, S], F32, tag="octx")
                nc.vector.tensor_copy(o[:], ctx_sb[h][:])
                nc.sync.dma_start(out_d[h * 64:(h + 1) * 64, 0:D], o[:, 0:D])
            return _finalize(ctx, nc, key)

        # ---- phase 4: output projection + residual + LayerNorm ----
        for s in range(S_CH):
            ps = psum.tile([128, S], F32, tag="big")
            for n0, nsz in VSPL:
                for h in range(H):
                    nc.tensor.matmul(ps[:, n0:n0 + nsz],
                                     ctx_sb[h][:, s * 128:(s + 1) * 128],
                                     wo_sb[h][:, n0:n0 + nsz],
                                     start=(h == 0), stop=False)
                nc.tensor.matmul(ps[:, n0:n0 + nsz],
                                 ones_row[0:1, 0:128],
                                 bo_sb[0:1, n0:n0 + nsz],
                                 start=False, stop=True)
            xn_t = xnpool.tile([128, D], F32, tag="xn")
            nc.sync.dma_start(xn_t[:], xn_d[s * 128:(s + 1) * 128, :])
            x = lnpool.tile([128, D], F32, tag="x")
            sacc = mpool.tile([128, 1], F32, tag="sacc")
            nc.vector.scalar_tensor_tensor(
                x[:], ps[:, 0:D], 1.0, xn_t[:],
                op0=ALU.mult, op1=ALU.add, accum_out=sacc[:])
            mu = mpool.tile([128, 1], F32, tag="mu")
            nc.vector.tensor_scalar_mul(mu[:], sacc[:], 1.0 / D)
            sq = lnpool.tile([128, D], F32, tag="scr", bufs=3)
            vacc = mpool.tile([128, 1], F32, tag="vacc")
            nc.vector.scalar_tensor_tensor(sq[:], x[:], mu[:], x[:],
                                           op0=ALU.subtract, op1=ALU.mult,
                                           accum_out=vacc[:])
            var_t = mpool.tile([128, 1], F32, tag="var")
            nc.vector.tensor_scalar(var_t[:], vacc[:], 1.0 / D, EPS,
                                    op0=ALU.mult, op1=ALU.add)
            sd = mpool.tile([128, 1], F32, tag="sd")
            nc.scalar.activation(sd[:], var_t[:], AF.Sqrt)
            rstd = mpool.tile([128, 1], F32, tag="rstd")
            nc.vector.reciprocal(rstd[:], sd[:])
            y = lnpool.tile([128, D], F32, tag="y")
            nc.vector.tensor_scalar(y[:], x[:], mu[:], rstd[:],
                                    op0=ALU.subtract, op1=ALU.mult)
            g = lnpool.tile([128, D], F32, tag="scr", bufs=3)
            nc.vector.scalar_tensor_tensor(g[:], y[:], 1.0, gb_sb[:],
                                           op0=ALU.mult, op1=ALU.mult)
            o = lnpool.tile([128, D], F32, tag="scr", bufs=3)
            nc.vector.tensor_tensor(o[:], g[:], bb_sb[:], ALU.add)
            nc.sync.dma_start(out_d[s * 128:(s + 1) * 128, :], o[:])

    return _finalize(ctx, nc, key)


def prep_inputs(hidden_states, attention_mask, head_mask, Wq, bq, Wk, bk,
                Wv, bv, Wo, bo, gamma, beta):
    """Host-side shard + layout prep. Returns per-core input maps."""
    bf = ml_dtypes.bfloat16
    hidden = np.asarray(hidden_states, np.float32)
    hm = np.asarray(head_mask, np.float32)
    hm_dv = np.repeat(hm, HD)  # per dv column
    wqt = np.ascontiguousarray(np.asarray(Wq, np.float32).T).astype(bf)
    wkt = np.ascontiguousarray(np.asarray(Wk, np.float32).T).astype(bf)
    wvt = np.ascontiguousarray(np.asarray(Wv, np.float32).T * hm_dv[None, :]).astype(bf)
    wot = np.ascontiguousarray(np.asarray(Wo, np.float32).T).astype(bf)
    bqr = np.asarray(bq, np.float32).reshape(1, D).astype(bf)
    bkr = np.asarray(bk, np.float32).reshape(1, D).astype(bf)
    bvr = (np.asarray(bv, np.float32) * hm_dv).reshape(1, D).astype(bf)
    bor = np.asarray(bo, np.float32).reshape(1, D).astype(bf)
    gammab = np.ascontiguousarray(
        np.broadcast_to(np.asarray(gamma, np.float32), (128, D)))
    betab = np.ascontiguousarray(
        np.broadcast_to(np.asarray(beta, np.float32), (128, D)))
    am = np.asarray(attention_mask, np.float32).reshape(B, S)

    in_maps = []
    for b in range(B):
        maskc = np.ascontiguousarray(am[b].reshape(S_CH, 128).T)  # [128, 8]
        in_maps.append({
            "xt": np.ascontiguousarray(hidden[b].T).astype(bf),
            "xn": np.ascontiguousarray(hidden[b]),
            "wqt": wqt, "wkt": wkt, "wvt": wvt, "wot": wot,
            "bqr": bqr, "bkr": bkr, "bvr": bvr, "bor": bor,
            "maskc": maskc, "gammab": gammab, "betab": betab,
        })
    return in_maps


def run(in_maps, trace=False, trace_cores=None, phases=4):
    _ensure_path()
    from concourse.bass_utils import run_bass_kernel_spmd
    nc = build_program(phases)
    return run_bass_kernel_spmd(nc, in_maps, list(range(NCORES)), trace=trace,
                                trace_cores=trace_cores)


def kernel(**inputs) -> np.ndarray:
    in_maps = prep_inputs(**inputs)
    res = run(in_maps)
    out = np.stack([res.results[b]["out"] for b in range(B)], axis=0)
    return out.astype(np.float32)


# revision 36
# speedup vs baseline: 1.3873x; 1.0564x over previous
"""BERT attention layer (nn_BertAttention) as a Bass/Tile kernel on 8 trn2 cores.

Sharding: data-parallel over batch (B=8 -> 1 batch element per core, no
collectives). Each core computes QKV projections, per-head attention,
masked output projection, residual + LayerNorm for its batch element.

Layout strategy (per core, S=1024, D=768, H=12, HD=64):
  - hidden^T (x_t [D, S]) feeds projections; Q,K produced transposed
    (q_t/k_t [D, S], head pair per 128-partition chunk), V natural [S, D].
  - scores computed transposed: s_t[k, q] = K @ Q^T per head. ACT exp
    folds the 1/sqrt(HD) scale and the per-k attention-mask bias.
  - PV: ctx^T[d, q] with e_t as moving operand; a concurrent all-ones
    [128,32] matmul in a disjoint column group produces the softmax
    denominator rows for free.
  - denominator reciprocal: one row is bounced through DRAM into a
    [128, 8] partition spread (so the iterative-divide DVE reciprocal
    touches 8 elems/lane instead of 1024), then broadcast back to
    [64, S] via a zero-stride DRAM read for the normalization multiply.
  - out-projection accumulates all 12 heads (+bias row) into psum per
    q-chunk; residual + LayerNorm fused on DVE with ACT ln/exp for
    rsqrt (keeps a single activation table set for the whole kernel).
  - head_mask is folded into Wv/bv host-side; biases enter as augmented
    rank-1 matmuls (ones x bias-row).
"""

import numpy as np
import ml_dtypes

B, S, D = 8, 1024, 768
H, HD = 12, 64
NCORES = 8
EPS = 1e-12
DI_CH = D // 128   # 6 contraction chunks
S_CH = S // 128    # 8 sequence chunks
PAIRS = H // 2     # 6 head pairs

_CACHE: dict = {}


def _ensure_path():
    import sys
    if "/opt/trn_rl_repo" not in sys.path:
        sys.path.insert(0, "/opt/trn_rl_repo")


def _finalize(ctx, nc, key):
    """Close the TileContext (schedules), compile, cache."""
    ctx.close()
    nc.compile()
    _CACHE[key] = nc
    return nc


def build_program(phases=4):
    """Build (once) the Bass program shared by all cores."""
    key = ("nc", phases)
    if key in _CACHE:
        return _CACHE[key]
    _ensure_path()
    from contextlib import ExitStack
    import concourse.bass as bass
    import concourse.bacc as bacc
    import concourse.mybir as mybir
    import concourse.tile as tile

    F32 = mybir.dt.float32
    BF16 = mybir.dt.bfloat16
    AF = mybir.ActivationFunctionType
    ALU = mybir.AluOpType

    nc = bacc.Bacc("TRN2", target_bir_lowering=False, debug=False)

    xt_d = nc.declare_dram_parameter("xt", [D, S], BF16, isOutput=False)
    xn_d = nc.declare_dram_parameter("xn", [S, D], F32, isOutput=False)
    wqt_d = nc.declare_dram_parameter("wqt", [D, D], BF16, isOutput=False)
    wkt_d = nc.declare_dram_parameter("wkt", [D, D], BF16, isOutput=False)
    wvt_d = nc.declare_dram_parameter("wvt", [D, D], BF16, isOutput=False)
    wot_d = nc.declare_dram_parameter("wot", [D, D], BF16, isOutput=False)
    bq_d = nc.declare_dram_parameter("bqr", [1, D], BF16, isOutput=False)
    bk_d = nc.declare_dram_parameter("bkr", [1, D], BF16, isOutput=False)
    bv_d = nc.declare_dram_parameter("bvr", [1, D], BF16, isOutput=False)
    bo_d = nc.declare_dram_parameter("bor", [1, D], BF16, isOutput=False)
    mask_d = nc.declare_dram_parameter("maskc", [128, S_CH], F32, isOutput=False)
    gb_d = nc.declare_dram_parameter("gammab", [128, D], F32, isOutput=False)
    bb_d = nc.declare_dram_parameter("betab", [128, D], F32, isOutput=False)
    out_d = nc.declare_dram_parameter("out", [S, D], F32, isOutput=True)

    dn_b1 = nc.dram_tensor("dn_b1", [H, S], F32)   # denominator bounce
    dn_b2 = nc.dram_tensor("dn_b2", [H, S], F32)   # reciprocal bounce

    with ExitStack() as ctx:
        tc = ctx.enter_context(tile.TileContext(nc))
        const = ctx.enter_context(tc.tile_pool(name="const", bufs=1))
        wpool = ctx.enter_context(tc.tile_pool(name="w", bufs=1))
        apool = ctx.enter_context(tc.tile_pool(name="act", bufs=1))
        epool = ctx.enter_context(tc.tile_pool(name="e", bufs=12))
        xnpool = ctx.enter_context(tc.tile_pool(name="xn", bufs=2))
        lnpool = ctx.enter_context(tc.tile_pool(name="ln", bufs=2))
        mpool = ctx.enter_context(tc.tile_pool(name="mini", bufs=4))
        rpool = ctx.enter_context(tc.tile_pool(name="rp", bufs=2))
        psum = ctx.enter_context(tc.tile_pool(name="ps", bufs=2, space="PSUM"))

        # ---- constants / small tensors ----
        ones_row = const.tile([1, S], BF16, tag="ones_row")
        nc.vector.memset(ones_row[:], 1.0)
        mask_sb = const.tile([128, S_CH], F32, tag="mask")
        nc.sync.dma_start(mask_sb[:], mask_d[:])
        gb_sb = const.tile([128, D], F32, tag="gb")
        nc.sync.dma_start(gb_sb[:], gb_d[:])
        bb_sb = const.tile([128, D], F32, tag="bb")
        nc.sync.dma_start(bb_sb[:], bb_d[:])
        bq_sb = const.tile([1, D], BF16, tag="bq")
        nc.sync.dma_start(bq_sb[:], bq_d[:])
        bk_sb = const.tile([1, D], BF16, tag="bk")
        nc.sync.dma_start(bk_sb[:], bk_d[:])
        bv_sb = const.tile([1, D], BF16, tag="bv")
        nc.sync.dma_start(bv_sb[:], bv_d[:])
        bo_sb = const.tile([1, D], BF16, tag="bo")
        nc.sync.dma_start(bo_sb[:], bo_d[:])

        # ---- bulk input loads ----
        xt_sb = []
        for c in range(DI_CH):
            t = wpool.tile([128, S], BF16, tag=f"xt{c}")
            nc.sync.dma_start(t[:], xt_d[c * 128:(c + 1) * 128, :])
            xt_sb.append(t)
        wq_sb, wk_sb, wv_sb = [], [], []
        for name, dram, lst in (("wq", wqt_d, wq_sb), ("wk", wkt_d, wk_sb),
                                ("wv", wvt_d, wv_sb)):
            for c in range(DI_CH):
                t = wpool.tile([128, D], BF16, tag=f"{name}{c}")
                nc.sync.dma_start(t[:], dram[c * 128:(c + 1) * 128, :])
                lst.append(t)
        # Wo stored per head at partition base 0: all out-projection
        # matmuls then accumulate with a uniform contract base (mixing
        # contract bases 0/64 inside one psum accumulation group faults
        # the device).
        wo_sb = []
        for h in range(H):
            t = wpool.tile([64, D], BF16, tag=f"wo{h}")
            nc.sync.dma_start(t[:], wot_d[h * HD:(h + 1) * HD, :])
            wo_sb.append(t)

        VSPL = [(0, 512), (512, 256)]  # free-dim splits for D=768 outputs

        # ---- projection units (emitted as PE filler inside the head
        # pipeline so ACT exp work starts as early as possible) ----
        # v stored per head with an appended all-ones 65th column: the PV
        # matmul then produces the softmax denominator as psum row 64 of
        # the same accumulation group (no extra matmuls).
        v_sb = []
        for s in range(S_CH):
            vt = apool.tile([128, H * (HD + 1)], BF16, tag=f"v{s}", name=f"v{s}")
            vt3 = vt.rearrange("p (h w) -> p h w", h=H)
            nc.vector.memset(vt3[:, :, HD:HD + 1], 1.0)
            v_sb.append(vt)
        q_sb, k_sb = [], []
        for which, lst in (("q", q_sb), ("k", k_sb)):
            for c in range(DI_CH):
                t = apool.tile([128, S], BF16, tag=f"{which}t{c}",
                               name=f"{which}t{c}")
                lst.append(t)

        def v_unit(s):
            vt3 = v_sb[s].rearrange("p (h w) -> p h w", h=H)
            ps = psum.tile([128, S], F32, tag="big", name="psv")
            for n0, nsz in VSPL:
                for di in range(DI_CH):
                    nc.tensor.matmul(ps[:, n0:n0 + nsz],
                                     xt_sb[di][:, s * 128:(s + 1) * 128],
                                     wv_sb[di][:, n0:n0 + nsz],
                                     start=(di == 0), stop=False)
                nc.tensor.matmul(ps[:, n0:n0 + nsz],
                                 ones_row[0:1, 0:128],
                                 bv_sb[0:1, n0:n0 + nsz],
                                 start=False, stop=True)
            nc.vector.tensor_copy(
                vt3[:, :, 0:HD],
                ps[:, 0:D].rearrange("p (h w) -> p h w", h=H))

        def qk_unit(which, c):
            w_sb, b_sb, t = ((wq_sb, bq_sb, q_sb[c]) if which == "q"
                             else (wk_sb, bk_sb, k_sb[c]))
            ps = psum.tile([128, S], F32, tag="big", name="psqk")
            for n in range(2):
                nsl = slice(n * 512, (n + 1) * 512)
                for di in range(DI_CH):
                    nc.tensor.matmul(ps[:, nsl],
                                     w_sb[di][:, c * 128:(c + 1) * 128],
                                     xt_sb[di][:, nsl],
                                     start=(di == 0), stop=False)
                nc.tensor.matmul(ps[:, nsl],
                                 b_sb[0:1, c * 128:(c + 1) * 128],
                                 ones_row[0:1, nsl],
                                 start=False, stop=True)
            nc.vector.tensor_copy(t[:], ps[:])

        # ---- attention with interleaved projection fillers ----
        ctx_sb = []
        for h in range(H):
            ctx_t = apool.tile([64, S], BF16, tag=f"ctx{h}", name=f"ctx{h}")
            ctx_sb.append(ctx_t)
        e_by_head = {}

        def emit_scores(h, c):
            p, half = h // 2, h % 2
            rlo, rhi = (0, 64) if half == 0 else (64, 128)
            ss = psum.tile([128, S], F32, tag="ss", name="ss")
            for n in range(2):
                nsl = slice(n * 512, (n + 1) * 512)
                nc.tensor.matmul(ss[:, nsl],
                                 k_sb[p][rlo:rhi, c * 128:(c + 1) * 128],
                                 q_sb[p][rlo:rhi, nsl],
                                 start=True, stop=True)
            e = epool.tile([128, S], BF16, tag="e", name="e")
            nc.scalar.activation(e[:], ss[:], AF.Exp,
                                 bias=mask_sb[:, c:c + 1], scale=0.125)
            e_by_head[h].append(e)

        def emit_pv(h, ct, c):
            st, sp = (c == 0), (c == S_CH - 1)
            for n in range(2):
                nsl = slice(n * 512, (n + 1) * 512)
                nc.tensor.matmul(ct[0:HD + 1, nsl],
                                 v_sb[c][:, h * (HD + 1):(h + 1) * (HD + 1)],
                                 e_by_head[h][c][:, nsl], start=st, stop=sp)

        def emit_norm(h, ct):
            # denominator -> DRAM -> [128, 8] spread -> reciprocal ->
            # DRAM -> zero-stride broadcast [64, S] -> normalize
            dcp = rpool.tile([1, S], F32, tag="dcp", bufs=1, name="dcp")
            nc.vector.tensor_copy(dcp[:], ct[HD:HD + 1, :])
            nc.sync.dma_start(dn_b1.ap()[h:h + 1, :], dcp[:])
            rs = rpool.tile([128, S // 128], F32, tag="rs", name="rs")
            nc.sync.dma_start(
                rs[:], dn_b1.ap()[h:h + 1, :].rearrange(
                    "one (p j) -> (one p) j", p=128))
            rc = rpool.tile([128, S // 128], F32, tag="rc", name="rc")
            nc.vector.reciprocal(rc[:], rs[:])
            nc.sync.dma_start(
                dn_b2.ap()[h:h + 1, :].rearrange(
                    "one (p j) -> (one p) j", p=128), rc[:])
            bc = rpool.tile([64, S], F32, tag="bc", bufs=1, name="bc")
            nc.sync.dma_start(bc[:], bass.AP(dn_b2, h * S, [[0, 64], [1, S]]))
            nc.vector.tensor_tensor(ctx_sb[h][:], ct[0:HD, :],
                                    bc[:], ALU.mult)

        qk_unit("q", 0)
        qk_unit("k", 0)
        fillers_by_head = {
            0: [lambda s=s: v_unit(s) for s in range(S_CH)]
               + [lambda: qk_unit("q", 1), lambda: qk_unit("k", 1)],
        }
        for c in range(2, DI_CH):
            fillers_by_head[c - 1] = [lambda c=c: qk_unit("q", c),
                                      lambda c=c: qk_unit("k", c)]

        ct_prev = None
        for h in range(H):
            e_by_head[h] = []
            fillers = fillers_by_head.get(h, [])
            done = 0
            for c in range(S_CH):
                emit_scores(h, c)
                if h > 0:
                    emit_pv(h - 1, ct_prev, c)
                want = (len(fillers) * (c + 1) + S_CH - 1) // S_CH
                while done < want:
                    fillers[done]()
                    done += 1
            if h > 0:
                emit_norm(h - 1, ct_prev)
                e_by_head.pop(h - 1)
            ct_prev = psum.tile([128, S], F32, tag="big", name="ct")
        for c in range(S_CH):
            emit_pv(H - 1, ct_prev, c)
        emit_norm(H - 1, ct_prev)

        if phases < 4:
            for h in range(H):
                o = lnpool.tile([64, S], F32, tag="octx")
                nc.vector.tensor_copy(o[:], ctx_sb[h][:])
                nc.sync.dma_start(out_d[h * 64:(h + 1) * 64, 0:D], o[:, 0:D])
            return _finalize(ctx, nc, key)

        # ---- phase 4: output projection + residual + LayerNorm ----
        for s in range(S_CH):
            ps = psum.tile([128, S], F32, tag="big")
            for n0, nsz in VSPL:
                for h in range(H):
                    nc.tensor.matmul(ps[:, n0:n0 + nsz],
                                     ctx_sb[h][:, s * 128:(s + 1) * 128],
                                     wo_sb[h][:, n0:n0 + nsz],
                                     start=(h == 0), stop=False)
                nc.tensor.matmul(ps[:, n0:n0 + nsz],
                                 ones_row[0:1, 0:128],
                                 bo_sb[0:1, n0:n0 + nsz],
                                 start=False, stop=True)
            xn_t = xnpool.tile([128, D], F32, tag="xn")
            nc.sync.dma_start(xn_t[:], xn_d[s * 128:(s + 1) * 128, :])
            x = lnpool.tile([128, D], F32, tag="x")
            sacc = mpool.tile([128, 1], F32, tag="sacc")
            nc.vector.scalar_tensor_tensor(
                x[:], ps[:, 0:D], 1.0, xn_t[:],
                op0=ALU.mult, op1=ALU.add, accum_out=sacc[:])
            mu = mpool.tile([128, 1], F32, tag="mu")
            nc.vector.tensor_scalar_mul(mu[:], sacc[:], 1.0 / D)
            sq = lnpool.tile([128, D], F32, tag="scr", bufs=3)
            vacc = mpool.tile([128, 1], F32, tag="vacc")
            nc.vector.scalar_tensor_tensor(sq[:], x[:], mu[:], x[:],
                                           op0=ALU.subtract, op1=ALU.mult,
                                           accum_out=vacc[:])
            var_t = mpool.tile([128, 1], F32, tag="var")
            nc.vector.tensor_scalar(var_t[:], vacc[:], 1.0 / D, EPS,
                                    op0=ALU.mult, op1=ALU.add)
            sd = mpool.tile([128, 1], F32, tag="sd")
            nc.scalar.activation(sd[:], var_t[:], AF.Sqrt)
            rstd = mpool.tile([128, 1], F32, tag="rstd")
            nc.vector.reciprocal(rstd[:], sd[:])
            y = lnpool.tile([128, D], F32, tag="y")
            nc.vector.tensor_scalar(y[:], x[:], mu[:], rstd[:],
                                    op0=ALU.subtract, op1=ALU.mult)
            g = lnpool.tile([128, D], F32, tag="scr", bufs=3)
            nc.vector.scalar_tensor_tensor(g[:], y[:], 1.0, gb_sb[:],
                                           op0=ALU.mult, op1=ALU.mult)
            o = lnpool.tile([128, D], F32, tag="scr", bufs=3)
            nc.vector.tensor_tensor(o[:], g[:], bb_sb[:], ALU.add)
            nc.sync.dma_start(out_d[s * 128:(s + 1) * 128, :], o[:])

    return _finalize(ctx, nc, key)


def prep_inputs(hidden_states, attention_mask, head_mask, Wq, bq, Wk, bk,
                Wv, bv, Wo, bo, gamma, beta):
    """Host-side shard + layout prep. Returns per-core input maps."""
    bf = ml_dtypes.bfloat16
    hidden = np.asarray(hidden_states, np.float32)
    hm = np.asarray(head_mask, np.float32)
    hm_dv = np.repeat(hm, HD)  # per dv column
    wqt = np.ascontiguousarray(np.asarray(Wq, np.float32).T).astype(bf)
    wkt = np.ascontiguousarray(np.asarray(Wk, np.float32).T).astype(bf)
    wvt = np.ascontiguousarray(np.asarray(Wv, np.float32).T * hm_dv[None, :]).astype(bf)
    wot = np.ascontiguousarray(np.asarray(Wo, np.float32).T).astype(bf)
    bqr = np.asarray(bq, np.float32).reshape(1, D).astype(bf)
    bkr = np.asarray(bk, np.float32).reshape(1, D).astype(bf)
    bvr = (np.asarray(bv, np.float32) * hm_dv).reshape(1, D).astype(bf)
    bor = np.asarray(bo, np.float32).reshape(1, D).astype(bf)
    gammab = np.ascontiguousarray(
        np.broadcast_to(np.asarray(gamma, np.float32), (128, D)))
    betab = np.ascontiguousarray(
        np.broadcast_to(np.asarray(beta, np.float32), (128, D)))
    am = np.asarray(attention_mask, np.float32).reshape(B, S)

    in_maps = []
    for b in range(B):
        maskc = np.ascontiguousarray(am[b].reshape(S_CH, 128).T)  # [128, 8]
        in_maps.append({
            "xt": np.ascontiguousarray(hidden[b].T).astype(bf),
            "xn": np.ascontiguousarray(hidden[b]),
            "wqt": wqt, "wkt": wkt, "wvt": wvt, "wot": wot,
            "bqr": bqr, "bkr": bkr, "bvr": bvr, "bor": bor,
            "maskc": maskc, "gammab": gammab, "betab": betab,
        })
    return in_maps


def run(in_maps, trace=False, trace_cores=None, phases=4):
    _ensure_path()
    from concourse.bass_utils import run_bass_kernel_spmd
    nc = build_program(phases)
    return run_bass_kernel_spmd(nc, in_maps, list(range(NCORES)), trace=trace,
                                trace_cores=trace_cores)


def kernel(**inputs) -> np.ndarray:
    in_maps = prep_inputs(**inputs)
    res = run(in_maps)
    out = np.stack([res.results[b]["out"] for b in range(B)], axis=0)
    return out.astype(np.float32)
